# revision 11
# baseline (speedup 1.0000x reference)
"""Bass/Trainium2 kernel for elementwise Bessel J2 (nn_BesselFunction).

Input:  x float32 [64, 1048576], values in [0, 30)
Output: J2(x) float32 [64, 1048576] (matches the NR-rational reference to
        ~2.5e-6 absolute; harness gate is rel 2e-2)

Sharding: trivially data-parallel; row-block shard across 8 NeuronCores.
Each core sees a [128, 65536] view of its 8-row slice.

v3: three engines in parallel per tile (vs v1's all-DVE 20-op pipeline).
Measured per-pass costs (65536 elems/core): DVE custom op 44us, Pool
tensor_scalar 114us, ACT activation 148us — so the bulk stays on DVE and
the slow-but-parallel engines each take work that would cost DVE more:
  DVE  (12 ops): recip, phase poly, round, Cody-Waite, tau affine,
                 amplitude poly (deg-9 in normalized r), small-branch poly
                 (deg-7 in x^2), predicated merge
  Pool (2 ops):  branch mask, final A*sin multiply
  ACT  (1 op):   sin spline (exact on [-pi,pi], ~1.5e-7) with -3pi/4 bias
Branch split moved from x=8 to x=6 so the small branch fits deg-7 in y=x^2
(3 DVE ops, no variable shift needed).

Math (x >= 6):  J2 = A(r) * sin(x - 3pi/4 + c(r)), r = 1/x
  c(r) = g0 r + g1 r^3 + g2 r^5           (minimax 4e-7 rad)
  A as deg-9 poly in tau = (r - MR)*SR    (minimax 4e-7)
  range reduction: k = round((xp - 3pi/4)/2pi) via magic-number add,
  th = xp - 2pi k (Cody-Waite 3-term), sin arg = th - 3pi/4 in [-pi,pi].
x < 6:  J2 = deg-7 poly in y = x^2 (minimax 6.3e-7, fp32 Horner ~2.5e-6).
Dead lanes (x<6 in the big path) may hold Inf/NaN; they are never read.
"""

import os
import sys

import numpy as np

for _p in ("/opt/trn_rl_repo", os.path.expanduser("~/.axon_site/_ro/trn_rl_repo")):
    if os.path.isdir(_p) and _p not in sys.path:
        sys.path.insert(0, _p)

# ---------------------------------------------------------------- constants
S_SPLIT = 6.0
# big branch phase: c(r) = G0*r + G1*r^3 + G2*r^5
G0, G1, G2 = 1.875014567079868, -0.3558901259440472, -1.1221412598630833
# amplitude variable: tau = (r - MR) * SR, r in [1/30.3, 1/5.85]
MR = 0.10197173563510198
SR = 14.499386503067486
# amplitude deg-9 poly coeffs (c0..c9) in tau
ACOEF = (0.2572930317220268, 0.09042766690264227, -0.012352728864134119,
         0.005169206050372702, -0.002199842710291278, 0.0011517614542460057,
         -0.00023685985874858856, -2.4615446058980436e-05,
         -0.00039264033844751446, 0.000288215505251411)
# small branch deg-7 poly coeffs (q0..q7) in y = x^2
QCOEF = (5.683716837734387e-07, 0.12499802019085911, -0.01041553497446274,
         0.00032527387571248815, -5.398758001773031e-06,
         5.494493263266567e-08, -3.5137150707190383e-10,
         1.1502952640315885e-12)

BIAS_3PI4 = 2.3561944901923448   # 3*pi/4 (ACT sin bias)
INV_2PI = 0.15915494309189535
MAGIC = 12582912.0               # 1.5 * 2^23
TWO_PI = 6.283185307179586
CW1 = 6.28125
CW2 = float(np.float32(TWO_PI - CW1))
CW3 = float(np.float32(TWO_PI - CW1 - np.float64(np.float32(TWO_PI - CW1))))

P = 128
COLS = 65536          # per-core elements / 128 partitions
FREE = 1024           # tile free dim
N_CORES = 8

_CACHE: dict = {}


def _register_custom_ops():
    from concourse import dve_ops
    from concourse.dve_spec import Spec, Src0, Src1, C0, C1, C2, sq, lower, _has_src1
    from concourse.dve_uop import DveOpSpec

    def register_op(name, spec):
        for op in dve_ops.OPS:
            if op.name == name:
                return op
        row = max(dve_ops._SUB_OPCODE_FOR_NAME.values()) + 1
        assert row < 0x20, "out of custom-DVE opcode rows"
        dve_ops._SUB_OPCODE_FOR_NAME[name] = row
        shas = {}
        for ver in ("v3", "v4"):
            try:
                s = DveOpSpec(name=name, opcode=row, uops=lower(spec, ver=ver),
                              rd1_en=_has_src1(spec))
                shas[ver] = s.sha(ver)
            except Exception:
                if ver == "v3":
                    raise
        op = dve_ops.DveOp(name, spec, subdim=False, uops_sha=shas)
        dve_ops.OPS.append(op)
        dve_ops.CUSTOM_DVE_SPECS[name] = spec
        return op

    ops = {}
    # ((c0*t + c1)*t + c2)*t : top of a deg-9 Horner chain (no const term)
    ops["POLY3T"] = register_op("J2_POLY3T", Spec(
        body=((C0 * Src0 + C1) * Src0 + C2) * Src0,
        reference=lambda in0, in1, c0, c1, c2: ((c0 * in0 + c1) * in0 + c2) * in0,
    ))
    # ((w + c0)*t + c1)*t + c2 : two more Horner degrees
    ops["HORNER2"] = register_op("J2_HORNER2", Spec(
        body=((Src0 + C0) * Src1 + C1) * Src1 + C2,
        reference=lambda in0, in1, c0, c1, c2: ((in0 + c0) * in1 + c1) * in1 + c2,
    ))
    # x + ((c0*u + c1)*u + c2)*r, u = r^2 : phase with odd correction poly
    ops["PHASE"] = register_op("J2_PHASE", Spec(
        body=Src1 + ((C0 * sq(Src0) + C1) * sq(Src0) + C2) * Src0,
        reference=lambda in0, in1, c0, c1, c2:
            in1 + ((c0 * in0 * in0 + c1) * (in0 * in0) + c2) * in0,
    ))
    # ((c0*y + c1)*y + c2)*y, y = x^2 : top of small-branch chain
    ops["AMP3"] = register_op("J2_AMP3", Spec(
        body=((C0 * sq(Src0) + C1) * sq(Src0) + C2) * sq(Src0),
        reference=lambda in0, in1, c0, c1, c2:
            ((c0 * in0 * in0 + c1) * (in0 * in0) + c2) * (in0 * in0),
    ))
    # ((w + c0)*y + c1)*y + c2, y = sq(Src1) : two more y-degrees
    ops["H2SQ"] = register_op("J2_H2SQ", Spec(
        body=((Src0 + C0) * sq(Src1) + C1) * sq(Src1) + C2,
        reference=lambda in0, in1, c0, c1, c2:
            ((in0 + c0) * (in1 * in1) + c1) * (in1 * in1) + c2,
    ))
    # ((x*c0 + c1) + c2) - c2 : round((xp - 3pi/4)/2pi) via magic add
    ops["ROUND"] = register_op("J2_ROUND", Spec(
        body=((Src0 * C0 + C1) + C2) - C2,
        reference=lambda in0, in1, c0, c1, c2:
            np.float32(np.float32(np.float32(np.float32(in0) * np.float32(c0))
                                  + np.float32(c1)) + np.float32(c2))
            - np.float32(c2),
    ))
    # plain elementwise product (keeps the output path on one engine)
    ops["MUL"] = register_op("J2_MUL", Spec(
        body=Src0 * Src1,
        reference=lambda in0, in1, c0, c1, c2: in0 * in1,
    ))
    return ops


def _build_program(repeat: int = 1, free: int = FREE):
    key = (repeat, free)
    if key in _CACHE:
        return _CACHE[key]

    from contextlib import ExitStack, nullcontext

    import concourse.bacc as bacc
    import concourse.bass as bass
    import concourse.tile as tile
    from concourse import mybir

    ops = _register_custom_ops()
    f32 = mybir.dt.float32
    ALU = mybir.AluOpType
    AF = mybir.ActivationFunctionType
    nt = COLS // free

    nc = bacc.Bacc("TRN2", target_bir_lowering=False, debug=False)
    # const AP for the ACT sin bias
    for v in (-BIAS_3PI4,):
        t = nc.alloc_sbuf_tensor(f"const-f32-{v}", [128, 1], f32)
        nc.gpsimd.memset(t.ap(), v)
        nc.const_aps.aps[(f32, v)] = t.ap()
    nc.all_engine_barrier()

    x_d = nc.dram_tensor("x", [P, COLS], f32, kind="ExternalInput")
    o_d = nc.dram_tensor("out", [P, COLS], f32, kind="ExternalOutput")
    x_ap = x_d.ap()
    o_ap = o_d.ap()

    cd = nc.vector._custom_dve

    with tile.TileContext(nc) as tc, ExitStack() as ctx:
        pools = {}
        bufn = {"xt": 4, "th": 3, "sb": 4, "w3": 3, "sm2": 3, "mk": 4,
                "ot": 4}
        for name in ("xt", "rf", "xp", "kk", "th", "sb",
                     "tr", "w0", "w1", "w2", "w3", "sm0", "sm1", "sm2",
                     "mk", "ot"):
            pools[name] = ctx.enter_context(
                tc.tile_pool(name=name, bufs=bufn.get(name, 2)))

        def pt(pool, tag=None, dtype=None):
            return pools[pool].tile([P, free], dtype or f32, name=tag or pool,
                                    tag=tag or pool)

        loop_cm = tc.For_i(0, repeat, 1) if repeat > 1 else nullcontext()
        with loop_cm:
          for i in range(nt):
            sl = bass.ts(i, free)
            xt = pt("xt")
            nc.sync.dma_start(xt[:], x_ap[:, sl])

            # ---- DVE: reciprocal + phase ----
            rf = pt("rf")
            nc.vector.reciprocal_approx_fast(out=rf[:], in_=xt[:])
            xp = pt("xp")
            cd(ops["PHASE"], out=xp[:], in0=rf[:], in1=xt[:],
               s0=G2, s1=G1, imm2=G0)

            # ---- DVE: k = round((xp - 3pi/4)/2pi); th = xp - 2pi*k ----
            kk = pt("kk")
            cd(ops["ROUND"], out=kk[:], in0=xp[:],
               s0=INV_2PI, s1=-0.375, imm2=MAGIC)
            th = pt("th")
            nc.vector.cody_waite_cascade(th[:], xp[:], kk[:], CW1, CW2, CW3)

            # ---- ACT: sb = sin(th - 3pi/4), arg in [-pi, pi] ----
            sb = pt("sb")
            nc.scalar.activation(sb[:], th[:], AF.Sin,
                                 bias=-BIAS_3PI4, scale=1.0)

            # ---- DVE: tau = (rf - MR)*SR, then amplitude poly ----
            tr = pt("tr")
            nc.vector.tensor_scalar(tr[:], rf[:], -MR, SR, ALU.add, ALU.mult)
            w = pt("w0")
            cd(ops["POLY3T"], out=w[:], in0=tr[:],
               s0=ACOEF[9], s1=ACOEF[8], imm2=ACOEF[7])
            for j, (b0, b1, b2) in enumerate(
                    ((ACOEF[6], ACOEF[5], 0.0),
                     (ACOEF[4], ACOEF[3], 0.0),
                     (ACOEF[2], ACOEF[1], ACOEF[0]))):
                w2 = pt(f"w{j + 1}")
                cd(ops["HORNER2"], out=w2[:], in0=w[:], in1=tr[:],
                   s0=b0, s1=b1, imm2=b2)
                w = w2

            # ---- DVE: small branch poly in y = x^2 ----
            sm = pt("sm0")
            cd(ops["AMP3"], out=sm[:], in0=xt[:],
               s0=QCOEF[7], s1=QCOEF[6], imm2=QCOEF[5])
            for j, (b0, b1, b2) in enumerate(
                    ((QCOEF[4], QCOEF[3], 0.0),
                     (QCOEF[2], QCOEF[1], QCOEF[0]))):
                sm2 = pt(f"sm{j + 1}")
                cd(ops["H2SQ"], out=sm2[:], in0=sm[:], in1=xt[:],
                   s0=b0, s1=b1, imm2=b2)
                sm = sm2

            # ---- Pool: mask ; DVE: final multiply + merge ----
            mk = pt("mk", dtype=mybir.dt.uint8)
            nc.gpsimd.tensor_scalar(mk[:], xt[:], S_SPLIT, None, ALU.is_lt)
            ot = pt("ot")
            cd(ops["MUL"], out=ot[:], in0=w[:], in1=sb[:])
            nc.vector.copy_predicated(ot[:], mk[:], sm[:])
            nc.sync.dma_start(o_ap[:, sl], ot[:])

    nc.compile()
    _CACHE[key] = {"nc": nc}
    return _CACHE[key]


def kernel(x: np.ndarray) -> np.ndarray:
    from concourse import bass_utils

    prog = _build_program()
    x = np.asarray(x, dtype=np.float32)
    rows = x.shape[0] // N_CORES
    in_maps = [
        {"x": np.ascontiguousarray(
            x[rows * k: rows * (k + 1)].reshape(P, COLS))}
        for k in range(N_CORES)
    ]
    res = bass_utils.run_bass_kernel_spmd(
        prog["nc"], in_maps, core_ids=list(range(N_CORES)))
    out = np.concatenate(
        [res.results[k]["out"].reshape(rows, -1) for k in range(N_CORES)], axis=0)
    return out.astype(np.float32)


# revision 14
# speedup vs baseline: 8.0595x; 8.0595x over previous
"""Bass/Trainium2 kernel for elementwise Bessel J2 (nn_BesselFunction).

Input:  x float32 [64, 1048576], values in [0, 30)
Output: J2(x) float32 [64, 1048576] (matches the NR-rational reference to
        ~2.5e-6 absolute; harness gate is rel 2e-2)

Sharding: trivially data-parallel; row-block shard across 8 NeuronCores.
Each core sees a [128, 65536] view of its 8-row slice.

v4: all-DVE 17-instruction pipeline (cross-engine offload to ACT/Pool
measured 2-2.5ms on this runtime — per-instruction cross-engine sync is
far more expensive than the DVE op it saves; DVE custom ops measured
44 us/full-data pass, so instruction count on one engine is the metric).
vs v1 (20 ops): branch split moved from x=8 to x=6 so the small branch fits
deg-7 in y=x^2 with no variable-shift op; sin is a deg-11 odd minimax in 2
fused ops (not 3); the -3pi/4 phase bias folds into the range reduction for
free via kappa = round((xp-3pi/4)/2pi) + 0.375 (kappa*CW1 is fp32-exact),
killing v1's explicit subtract; amplitude is a deg-9 poly in normalized
1/x (one stock affine + 4 fused ops) replacing v1's seed+2xNewton rsqrt
(6 ops).

Math (x >= 6):  J2 = A(r) * sin(x - 3pi/4 + c(r)), r = 1/x
  c(r) = g0 r + g1 r^3 + g2 r^5           (minimax 4e-7 rad)
  A as deg-9 poly in tau = (r - MR)*SR    (minimax 4e-7)
  range reduction: k = round((xp - 3pi/4)/2pi) via magic-number add,
  th = xp - 2pi k (Cody-Waite 3-term), sin arg = th - 3pi/4 in [-pi,pi].
x < 6:  J2 = deg-7 poly in y = x^2 (minimax 6.3e-7, fp32 Horner ~2.5e-6).
Dead lanes (x<6 in the big path) may hold Inf/NaN; they are never read.
"""

import os
import sys

import numpy as np

for _p in ("/opt/trn_rl_repo", os.path.expanduser("~/.axon_site/_ro/trn_rl_repo")):
    if os.path.isdir(_p) and _p not in sys.path:
        sys.path.insert(0, _p)

# ---------------------------------------------------------------- constants
S_SPLIT = 6.0
# big branch phase: c(r) = G0*r + G1*r^3 + G2*r^5
G0, G1, G2 = 1.875014567079868, -0.3558901259440472, -1.1221412598630833
# amplitude variable: tau = (r - MR) * SR, r in [1/30.3, 1/5.85]
MR = 0.10197173563510198
SR = 14.499386503067486
# amplitude deg-9 poly coeffs (c0..c9) in tau
ACOEF = (0.2572930317220268, 0.09042766690264227, -0.012352728864134119,
         0.005169206050372702, -0.002199842710291278, 0.0011517614542460057,
         -0.00023685985874858856, -2.4615446058980436e-05,
         -0.00039264033844751446, 0.000288215505251411)
# small branch deg-7 poly coeffs (q0..q7) in y = x^2
QCOEF = (5.683716837734387e-07, 0.12499802019085911, -0.01041553497446274,
         0.00032527387571248815, -5.398758001773031e-06,
         5.494493263266567e-08, -3.5137150707190383e-10,
         1.1502952640315885e-12)

# deg-11 odd sin: sin(t) ~ t*(SINC[0] + SINC[1] s + ... + SINC[5] s^5), s=t^2
SINC = (0.9999996087121806, -0.16666554514455598, 0.008332414168748812,
        -0.00019808903582239294, 2.699999332800843e-06,
        -2.037311695809314e-08)
BIAS_3PI4 = 2.3561944901923448   # 3*pi/4
INV_2PI = 0.15915494309189535
MAGIC = 12582912.0               # 1.5 * 2^23
TWO_PI = 6.283185307179586
CW1 = 6.28125
CW2 = float(np.float32(TWO_PI - CW1))
CW3 = float(np.float32(TWO_PI - CW1 - np.float64(np.float32(TWO_PI - CW1))))

P = 128
COLS = 65536          # per-core elements / 128 partitions
FREE = 1024           # tile free dim
N_CORES = 8

_CACHE: dict = {}


def _register_custom_ops():
    from concourse import dve_ops
    from concourse.dve_spec import (Spec, Src0, Src1, C0, C1, C2, C3, sq,
                                    lower, _has_src1)
    from concourse.dve_uop import DveOpSpec

    def register_op(name, spec):
        for op in dve_ops.OPS:
            if op.name == name:
                return op
        row = max(dve_ops._SUB_OPCODE_FOR_NAME.values()) + 1
        assert row < 0x20, "out of custom-DVE opcode rows"
        dve_ops._SUB_OPCODE_FOR_NAME[name] = row
        shas = {}
        for ver in ("v3", "v4"):
            try:
                s = DveOpSpec(name=name, opcode=row, uops=lower(spec, ver=ver),
                              rd1_en=_has_src1(spec))
                shas[ver] = s.sha(ver)
            except Exception:
                if ver == "v3":
                    raise
        op = dve_ops.DveOp(name, spec, subdim=False, uops_sha=shas)
        dve_ops.OPS.append(op)
        dve_ops.CUSTOM_DVE_SPECS[name] = spec
        return op

    ops = {}
    # ((c0*t + c1)*t + c2)*t : top of a deg-9 Horner chain (no const term)
    ops["POLY3T"] = register_op("J2_POLY3T", Spec(
        body=((C0 * Src0 + C1) * Src0 + C2) * Src0,
        reference=lambda in0, in1, c0, c1, c2: ((c0 * in0 + c1) * in0 + c2) * in0,
    ))
    # ((w + c0)*t + c1)*t + c2 : two more Horner degrees
    ops["HORNER2"] = register_op("J2_HORNER2", Spec(
        body=((Src0 + C0) * Src1 + C1) * Src1 + C2,
        reference=lambda in0, in1, c0, c1, c2: ((in0 + c0) * in1 + c1) * in1 + c2,
    ))
    # x + ((c0*u + c1)*u + c2)*r, u = r^2 : phase with odd correction poly
    ops["PHASE"] = register_op("J2_PHASE", Spec(
        body=Src1 + ((C0 * sq(Src0) + C1) * sq(Src0) + C2) * Src0,
        reference=lambda in0, in1, c0, c1, c2:
            in1 + ((c0 * in0 * in0 + c1) * (in0 * in0) + c2) * in0,
    ))
    # ((c0*y + c1)*y + c2)*y, y = x^2 : top of small-branch chain
    ops["AMP3"] = register_op("J2_AMP3", Spec(
        body=((C0 * sq(Src0) + C1) * sq(Src0) + C2) * sq(Src0),
        reference=lambda in0, in1, c0, c1, c2:
            ((c0 * in0 * in0 + c1) * (in0 * in0) + c2) * (in0 * in0),
    ))
    # ((w + c0)*y + c1)*y + c2, y = sq(Src1) : two more y-degrees
    ops["H2SQ"] = register_op("J2_H2SQ", Spec(
        body=((Src0 + C0) * sq(Src1) + C1) * sq(Src1) + C2,
        reference=lambda in0, in1, c0, c1, c2:
            ((in0 + c0) * (in1 * in1) + c1) * (in1 * in1) + c2,
    ))
    # (((x*c0 + c1) + c2) - c2) - c1 : kappa = round(x*c0 + c1) - c1, i.e.
    # round((xp - 3pi/4)/2pi) + 0.375 with c1 = -0.375 (reused for the shift)
    ops["ROUND2"] = register_op("J2_ROUND2", Spec(
        body=(((Src0 * C0 + C1) + C2) - C2) - C1,
        reference=lambda in0, in1, c0, c1, c2:
            np.float32(np.float32(np.float32(np.float32(np.float32(in0)
                * np.float32(c0)) + np.float32(c1)) + np.float32(c2))
                - np.float32(c2)) - np.float32(c1),
    ))
    # (((w + c0)*s + c1)*s + c2)*t, s = sq(Src1) : sin-chain tail (x theta)
    ops["H2SQM"] = register_op("J2_H2SQM", Spec(
        body=(((Src0 + C0) * sq(Src1) + C1) * sq(Src1) + C2) * Src1,
        reference=lambda in0, in1, c0, c1, c2:
            (((in0 + c0) * (in1 * in1) + c1) * (in1 * in1) + c2) * in1,
    ))
    # plain elementwise product (keeps the output path on one engine)
    ops["MUL"] = register_op("J2_MUL", Spec(
        body=Src0 * Src1,
        reference=lambda in0, in1, c0, c1, c2: in0 * in1,
    ))
    return ops


def _build_program(repeat: int = 1, free: int = FREE):
    key = (repeat, free)
    if key in _CACHE:
        return _CACHE[key]

    from contextlib import ExitStack, nullcontext

    import concourse.bacc as bacc
    import concourse.bass as bass
    import concourse.tile as tile
    from concourse import mybir

    ops = _register_custom_ops()
    f32 = mybir.dt.float32
    ALU = mybir.AluOpType
    AF = mybir.ActivationFunctionType
    nt = COLS // free

    nc = bacc.Bacc("TRN2", target_bir_lowering=False, debug=False)

    x_d = nc.dram_tensor("x", [P, COLS], f32, kind="ExternalInput")
    o_d = nc.dram_tensor("out", [P, COLS], f32, kind="ExternalOutput")
    x_ap = x_d.ap()
    o_ap = o_d.ap()

    cd = nc.vector._custom_dve

    with tile.TileContext(nc) as tc, ExitStack() as ctx:
        pools = {}
        for name in ("xt", "rf", "xp", "kk", "th", "s1", "sb",
                     "tr", "w0", "w1", "w2", "w3", "sm0", "sm1", "sm2",
                     "mk", "ot"):
            pools[name] = ctx.enter_context(tc.tile_pool(name=name, bufs=2))

        def pt(pool, tag=None, dtype=None):
            return pools[pool].tile([P, free], dtype or f32, name=tag or pool,
                                    tag=tag or pool)

        loop_cm = tc.For_i(0, repeat, 1) if repeat > 1 else nullcontext()
        with loop_cm:
          for i in range(nt):
            sl = bass.ts(i, free)
            xt = pt("xt")
            nc.sync.dma_start(xt[:], x_ap[:, sl])

            # ---- DVE: reciprocal + phase ----
            rf = pt("rf")
            nc.vector.reciprocal_approx_fast(out=rf[:], in_=xt[:])
            xp = pt("xp")
            cd(ops["PHASE"], out=xp[:], in0=rf[:], in1=xt[:],
               s0=G2, s1=G1, imm2=G0)

            # ---- DVE: kappa = round((xp-3pi/4)/2pi)+0.375; th in [-pi,pi] ----
            kk = pt("kk")
            cd(ops["ROUND2"], out=kk[:], in0=xp[:],
               s0=INV_2PI, s1=-0.375, imm2=MAGIC)
            th = pt("th")
            nc.vector.cody_waite_cascade(th[:], xp[:], kk[:], CW1, CW2, CW3)

            # ---- DVE: sb = sin(th), deg-11 odd minimax ----
            s1_ = pt("s1")
            cd(ops["AMP3"], out=s1_[:], in0=th[:],
               s0=SINC[5], s1=SINC[4], imm2=SINC[3])
            sb = pt("sb")
            cd(ops["H2SQM"], out=sb[:], in0=s1_[:], in1=th[:],
               s0=SINC[2], s1=SINC[1], imm2=SINC[0])

            # ---- DVE: tau = (rf - MR)*SR, then amplitude poly ----
            tr = pt("tr")
            nc.vector.tensor_scalar(tr[:], rf[:], -MR, SR, ALU.add, ALU.mult)
            w = pt("w0")
            cd(ops["POLY3T"], out=w[:], in0=tr[:],
               s0=ACOEF[9], s1=ACOEF[8], imm2=ACOEF[7])
            for j, (b0, b1, b2) in enumerate(
                    ((ACOEF[6], ACOEF[5], 0.0),
                     (ACOEF[4], ACOEF[3], 0.0),
                     (ACOEF[2], ACOEF[1], ACOEF[0]))):
                w2 = pt(f"w{j + 1}")
                cd(ops["HORNER2"], out=w2[:], in0=w[:], in1=tr[:],
                   s0=b0, s1=b1, imm2=b2)
                w = w2

            # ---- DVE: small branch poly in y = x^2 ----
            sm = pt("sm0")
            cd(ops["AMP3"], out=sm[:], in0=xt[:],
               s0=QCOEF[7], s1=QCOEF[6], imm2=QCOEF[5])
            for j, (b0, b1, b2) in enumerate(
                    ((QCOEF[4], QCOEF[3], 0.0),
                     (QCOEF[2], QCOEF[1], QCOEF[0]))):
                sm2 = pt(f"sm{j + 1}")
                cd(ops["H2SQ"], out=sm2[:], in0=sm[:], in1=xt[:],
                   s0=b0, s1=b1, imm2=b2)
                sm = sm2

            # ---- DVE: mask + final multiply + merge ----
            mk = pt("mk", dtype=mybir.dt.uint8)
            nc.vector.tensor_scalar(mk[:], xt[:], S_SPLIT, None, ALU.is_lt)
            ot = pt("ot")
            cd(ops["MUL"], out=ot[:], in0=w[:], in1=sb[:])
            nc.vector.copy_predicated(ot[:], mk[:], sm[:])
            nc.sync.dma_start(o_ap[:, sl], ot[:])

    nc.compile()
    _CACHE[key] = {"nc": nc}
    return _CACHE[key]


def kernel(x: np.ndarray) -> np.ndarray:
    from concourse import bass_utils

    prog = _build_program()
    x = np.asarray(x, dtype=np.float32)
    rows = x.shape[0] // N_CORES
    in_maps = [
        {"x": np.ascontiguousarray(
            x[rows * k: rows * (k + 1)].reshape(P, COLS))}
        for k in range(N_CORES)
    ]
    res = bass_utils.run_bass_kernel_spmd(
        prog["nc"], in_maps, core_ids=list(range(N_CORES)))
    out = np.concatenate(
        [res.results[k]["out"].reshape(rows, -1) for k in range(N_CORES)], axis=0)
    return out.astype(np.float32)
